# revision 6
# baseline (speedup 1.0000x reference)
# PointNet++ regressor on 8 NeuronCores.
# Data-parallel over the batch axis (B=32 -> 4 clouds per core), with
# cross-core reductions for the train-mode BatchNorm statistics (the only
# cross-cloud coupling in the model). Head (fc1/fc2 + BN over batch, fc3/fcn,
# tanh + normalize) is computed per shard with the same global-stat
# reductions; outputs are gathered to the full [32, 1500, 6] tensor.
#
# Robustness ladder: 8-device shard_map -> single-device jit -> CPU.
import numpy as np

BATCH = 32
NPTS = 2048
EPS_BN = 1e-5
OUT_VECTORS = 1500
N_CORES = 8


def _forward_factory(jnp, jax, axis_name=None):
    """Build the forward pass. If axis_name is set, BN statistics are
    all-reduced across the device axis (each device holds an equal batch
    shard), exactly reproducing global train-mode BN."""

    def gmean(x):
        if axis_name is None:
            return x
        return jax.lax.pmean(x, axis_name)

    def _sqrdist(a, b):
        return (jnp.sum(a * a, -1)[:, :, None] + jnp.sum(b * b, -1)[:, None, :]
                - 2.0 * jnp.einsum('bsc,bnc->bsn', a, b))

    def _fps(xyz, npoint):
        B, N, _ = xyz.shape

        def body(carry, _):
            dist, far = carry
            centroid = jnp.take_along_axis(xyz, far[:, None, None], axis=1)
            d = jnp.sum((xyz - centroid) ** 2, -1)
            dist = jnp.minimum(dist, d)
            return (dist, jnp.argmax(dist, -1).astype(jnp.int32)), far

        init = (jnp.full((B, N), 1e10, xyz.dtype), jnp.zeros((B,), jnp.int32))
        # unroll: same op sequence, 8x fewer dynamic loop trips (per-trip
        # dispatch overhead is significant on the neuron backend)
        _, idx = jax.lax.scan(body, init, None, length=npoint, unroll=8)
        return idx.T

    def _gather(points, idx):
        return jax.vmap(lambda p, i: p[i])(points, idx)

    def _ball_query(radius, nsample, xyz, new_xyz):
        # Equivalent to the reference's sort(idx)[..., :nsample]: the nsample
        # smallest entries of idx (in-ball point indices ascending, N = out of
        # ball), but via top_k of the negation — O(N log k) instead of a full
        # 2048-wide sort per center.
        N = xyz.shape[1]
        sq = _sqrdist(new_xyz, xyz)
        ar = jnp.arange(N, dtype=jnp.int32)[None, None, :]
        idx = jnp.where(sq > radius * radius, N, ar)
        neg, _ = jax.lax.top_k(-idx, nsample)
        idx = -neg  # ascending smallest-nsample
        first = idx[:, :, :1]
        return jnp.where(idx == N, jnp.broadcast_to(first, idx.shape), idx)

    def _mlp_bn_relu(x, layers):
        for p in layers:
            x = jnp.einsum('...c,cd->...d', x, p["w"]) + p["b"]
            ax = tuple(range(x.ndim - 1))
            m = gmean(x.mean(ax))
            v = gmean(((x - m) ** 2).mean(ax))
            x = (x - m) * jax.lax.rsqrt(v + EPS_BN) * p["gamma"] + p["beta"]
            x = jax.nn.relu(x)
        return x

    def _set_abstraction(xyz, points, npoint, radius, nsample, layers, group_all):
        if group_all:
            feats = xyz[:, None, :, :]
            if points is not None:
                feats = jnp.concatenate([feats, points[:, None, :, :]], -1)
            new_xyz = jnp.zeros((xyz.shape[0], 1, 3), xyz.dtype)
        else:
            fidx = _fps(xyz, npoint)
            new_xyz = _gather(xyz, fidx)
            gidx = _ball_query(radius, nsample, xyz, new_xyz)
            feats = _gather(xyz, gidx) - new_xyz[:, :, None, :]
            if points is not None:
                feats = jnp.concatenate([feats, _gather(points, gidx)], -1)
        feats = _mlp_bn_relu(feats, layers)
        return new_xyz, feats.max(axis=2)

    def _batchnorm1d(x, g, b):
        # full-batch stats, computed locally (x already holds the whole batch)
        m = x.mean(0)
        v = ((x - m) ** 2).mean(0)
        return (x - m) * jax.lax.rsqrt(v + EPS_BN) * g + b

    def forward(xyz, params):
        B = xyz.shape[0]
        pts = jnp.transpose(xyz, (0, 2, 1))
        l1x, l1p = _set_abstraction(pts, None, 512, 0.2, 32, params["sa1"], False)
        l2x, l2p = _set_abstraction(l1x, l1p, 128, 0.4, 64, params["sa2"], False)
        _, l3p = _set_abstraction(l2x, l2p, None, None, None, params["sa3"], True)
        gf = l3p.reshape(B, 1024)
        if axis_name is not None:
            # One tiny all-gather of the global features ([32,1024]); the head
            # is then computed replicated with fully local batch stats, which
            # replaces four per-layer stat all-reduces with one collective.
            gf = jax.lax.all_gather(gf, axis_name, axis=0, tiled=True)
        x = jax.nn.relu(_batchnorm1d(gf @ params["fc1_w"] + params["fc1_b"],
                                     params["bn1_g"], params["bn1_b"]))
        final = jax.nn.relu(_batchnorm1d(x @ params["fc2_w"] + params["fc2_b"],
                                         params["bn2_g"], params["bn2_b"]))
        trans = (final @ params["fc3_w"] + params["fc3_b"]).reshape(-1, OUT_VECTORS, 3)
        nrm = jnp.tanh(final @ params["fcn_w"] + params["fcn_b"]).reshape(-1, OUT_VECTORS, 3)
        nrm = nrm / jnp.maximum(jnp.linalg.norm(nrm, axis=-1, keepdims=True), 1e-12)
        out = jnp.concatenate([trans, nrm], -1)
        if axis_name is not None:
            # keep only this shard's clouds; shard_map reassembles the batch
            r = jax.lax.axis_index(axis_name)
            out = jax.lax.dynamic_slice_in_dim(out, r * B, B, axis=0)
        return out

    return forward


_JIT_CACHE = {}


def _run_sharded(xyz, params):
    import jax
    import jax.numpy as jnp
    from jax.sharding import Mesh, PartitionSpec as P
    try:
        from jax.experimental.shard_map import shard_map
    except ImportError:  # newer jax moved it
        from jax import shard_map

    fn = _JIT_CACHE.get("sharded")
    if fn is None:
        devs = jax.devices()
        if len(devs) < N_CORES:
            raise RuntimeError(f"need {N_CORES} devices, have {len(devs)}")
        mesh = Mesh(np.asarray(devs[:N_CORES]), ("b",))
        forward = _forward_factory(jnp, jax, axis_name="b")
        pspec = jax.tree.map(lambda _: P(), params)
        fn = jax.jit(shard_map(forward, mesh=mesh,
                               in_specs=(P("b"), pspec), out_specs=P("b"),
                               check_rep=False))
        _JIT_CACHE["sharded"] = fn
    out = fn(jnp.asarray(xyz), params)
    return np.asarray(out)


def _run_single(xyz, params):
    import jax
    import jax.numpy as jnp
    fn = _JIT_CACHE.get("single")
    if fn is None:
        fn = jax.jit(_forward_factory(jnp, jax, axis_name=None))
        _JIT_CACHE["single"] = fn
    out = fn(jnp.asarray(xyz), params)
    return np.asarray(out)


def kernel(xyz, params):
    xyz = np.asarray(xyz, dtype=np.float32)
    try:
        out = _run_sharded(xyz, params)
    except Exception:
        try:
            out = _run_single(xyz, params)
        except Exception:
            # last-resort CPU execution
            import jax
            import jax.numpy as jnp
            with jax.default_device(jax.devices("cpu")[0]):
                forward = _forward_factory(jnp, jax, axis_name=None)
                out = np.asarray(forward(jnp.asarray(xyz), params))
    return np.asarray(out, dtype=np.float32)
